# revision 18
# baseline (speedup 1.0000x reference)
"""Trainium2 Bass kernel for nn_ConvolutionLayer (FFT conv collapse), v4.

Math: reference computes
    u_fft = rfft(u); ev_fft = rfft(ev)
    p_fft = einsum('bi,kj->bkj', u_fft, ev_fft)      # sums u_fft over i!
    conv  = irfft(p_fft); result = einsum('bkl,k->bl', conv, lam)

The einsum has no shared index, so p_fft[b,k,j] = s_b * ev_fft[k,j] with
s_b = sum_i u_fft[b,i] = u[b,:] @ g   (g = fft(indicator of first L/2+1)).
irfft is R-linear, so with s_b = a_b + i*c_b:
    result[b,:] = a_b * w0 + c_b * w1
    w0 = lam @ ev                       (since irfft(rfft(e)) = e)
    w1 = irfft(i * rfft(w0))            (by linearity over k)
w1 via a 4-step Cooley-Tukey matmul-FFT (64x128 grid), the Hermitian
symbol (+i*sgn) applied mid-chain; everything bf16 (4-5e-3 rel err vs
the 2e-2 gate).

Schedule (per core, times approximate):
  sync  ring: EVA (lam windows + ev half 1), EVB (ev half 2), CB
              (FFT constants), then out-DMAs for pairs 0/2
  gpsimd ring: CM (early constants: F64S/I64/STK/GODD/masks), U
  scalar ring: out-DMAs for pairs 1/3 (act-table load owns it early)
  PE: 16 w0 matmuls (8 per EV chunk) -> acr -> fft stages 1..4
  DVE: u-dots early; cast, twiddle1, symbol, twiddle2, 4 final STTs
  Scalar engine: acrS copy + 4 tmpA = a_j*x products off PSUM
  GpSimd engine: Zim / Qim adds (parallel with DVE's Zre / Qre)

FFT stage tricks: stage1 is 1 matmul (rhs=[F64re|F64im]); stage2 is 3
matmuls into one PSUM tile (full-width F_re pass + two half-width F_im
accumulations) so no negated operand copy is needed; symbol is one wide
multiply producing doubled windows [Xre'|Xre'|Xim'|Xim']; stage3 output
is c-doubled so twiddle2 runs on 128 partitions; stage4 rhs takes
partition slices.  Batch (64) is sharded across 8 cores, 8 rows each;
the w0/w1 pipeline is tiny and computed redundantly (no collectives).
"""

import numpy as np
import ml_dtypes

_B, _K, _L = 64, 32, 8192
_NC = 8
_BS = _B // _NC  # 8 batch rows per core

_BF = ml_dtypes.bfloat16

# ---------------------------------------------------------------- constants


def _pack(parts):
    off, offs = 0, {}
    for name, arr in parts.items():
        offs[name] = (off, arr.shape[1])
        off += arr.shape[1]
    return np.ascontiguousarray(np.concatenate(list(parts.values()), axis=1)), offs


def _build_constants():
    L, N1, N2 = _L, 64, 128
    bf = lambda x: np.ascontiguousarray(np.asarray(x, np.float32).astype(_BF))

    def pad128(a):  # place 64-row constant in rows 0:64 of a 128-row block
        out = np.zeros((128, a.shape[1]), a.dtype)
        out[:64] = a
        return out

    # g[n] = sum_{i=0}^{L/2} e^{-2pi i n i / L}; re: 1 on even n (L/2+1 at 0),
    # im: -cot(pi n / L) on odd n.  U layout: U[16b+t, f] = u[b, 512t+f].
    ind = np.zeros(L)
    ind[: L // 2 + 1] = 1.0
    g = np.fft.fft(ind)
    gim = g.imag.astype(np.float32).reshape(16, 512)  # [t, f]
    godd = np.tile(gim[:, 1::2], (8, 1))  # (128, 256)

    m16 = np.zeros((128, 1), np.float32)  # l=0 correction 4096*u[b,0]
    m16[::16, 0] = 4096.0

    mask8 = np.zeros((128, 8), np.float32)
    for p in range(128):
        j = (p // 16) // 2
        mask8[p, 2 * j : 2 * j + 2] = 1.0
    stk = np.zeros((128, 128), np.float32)
    for p in range(128):
        stk[p, 64 * ((p // 16) % 2) : 64 * ((p // 16) % 2) + 64] = 1.0

    a_i = np.arange(N1)
    b_i = np.arange(N2)
    F64 = np.exp(-2j * np.pi * np.outer(a_i, a_i) / N1)  # [a, c]
    WT = np.exp(-2j * np.pi * np.outer(b_i, a_i) / L)  # [b=n2, c=k1]
    F128 = np.exp(-2j * np.pi * np.outer(b_i, b_i) / N2)  # [b, d]
    kk = a_i[None, :] + N1 * b_i[:, None]  # spectrum index (128 d, 64 c)
    sgn = np.where(
        (kk >= 1) & (kk <= L // 2 - 1), 1.0, np.where(kk > L // 2, -1.0, 0.0)
    )
    I128 = np.exp(+2j * np.pi * np.outer(b_i, b_i) / N2)  # [d, m2]
    Wi = np.exp(+2j * np.pi * np.outer(a_i, b_i) / L)  # [c, m2]
    I64 = np.exp(+2j * np.pi * np.outer(a_i, a_i) / N1) / L  # [c, m1]

    # CM: early constants (gpsimd queue, lands first)
    pcm = {
        "F64S": bf(pad128(np.hstack([F64.real, F64.imag]))),  # 128
        "I64_re2": bf(pad128(np.hstack([I64.real, I64.real]))),  # 128
        "I64_imN2": bf(pad128(np.hstack([-I64.imag, -I64.imag]))),  # 128
        "STK": bf(stk),  # 128
        "GODD": bf(godd),  # 256
        "MASK8": bf(mask8),  # 8
        "M16": bf(m16),  # 1
    }
    # CB: FFT constants (sync queue, behind the EV chunks)
    pcb = {
        "WQ": bf(np.hstack([WT.real, WT.imag, WT.imag, WT.real])),  # 256
        "F128_re": bf(F128.real),  # 128
        "F128_im": bf(F128.imag),  # 128
        "F128_imN": bf(-F128.imag),  # 128
        "SG4": bf(np.hstack([-sgn, -sgn, sgn, sgn])),  # 256
        "IA": bf(np.hstack([I128.real, I128.imag])),  # 256
        "IB": bf(np.hstack([-I128.imag, I128.real])),  # 256
        "WiQ2": bf(
            np.tile(np.hstack([Wi.real, Wi.imag, Wi.imag, Wi.real]), (2, 1))
        ),  # 512
    }
    CM, cm_off = _pack(pcm)
    CB, cb_off = _pack(pcb)
    return CM, cm_off, CB, cb_off


_CM, _CM_OFF, _CB, _CB_OFF = _build_constants()
_LAMB_W = 188
_EVA_W = _LAMB_W + 1024  # lam windows + ev columns 0:1024
_EVB_W = 512

# ---------------------------------------------------------------- bass build

_COMPILED = None


def _build_nc():
    import concourse.mybir as mybir
    import concourse.tile as tile
    from concourse import bacc

    f32 = mybir.dt.float32
    bf16 = mybir.dt.bfloat16
    Alu = mybir.AluOpType

    nc = bacc.Bacc(None)

    u_d = nc.declare_dram_parameter("u", [128, 512], bf16, isOutput=False)
    eva_d = nc.declare_dram_parameter("eva", [128, _EVA_W], bf16, isOutput=False)
    evb_d = nc.declare_dram_parameter("evb", [128, _EVB_W], bf16, isOutput=False)
    evc_d = nc.declare_dram_parameter("evc", [128, _EVB_W], bf16, isOutput=False)
    cm_d = nc.declare_dram_parameter("cm", list(_CM.shape), bf16, isOutput=False)
    cb_d = nc.declare_dram_parameter("cb", list(_CB.shape), bf16, isOutput=False)
    out_d = nc.declare_dram_parameter("out", [128, 512], bf16, isOutput=True)

    def cm(t, name, rows=None):
        off, w = _CM_OFF[name]
        return t[:, off : off + w] if rows is None else t[0:rows, off : off + w]

    def cb(t, name):
        off, w = _CB_OFF[name]
        return t[:, off : off + w]

    with tile.TileContext(nc) as tc:
        with (
            tc.tile_pool(name="const", bufs=1) as constp,
            tc.tile_pool(name="sb", bufs=1) as sb,
            tc.tile_pool(name="psx", bufs=1, space="PSUM") as psx,
            tc.tile_pool(name="psacr", bufs=1, space="PSUM") as psacr,
            tc.tile_pool(name="psyt", bufs=1, space="PSUM") as psyt,
            tc.tile_pool(name="psxt", bufs=1, space="PSUM") as psxt,
            tc.tile_pool(name="psgq", bufs=1, space="PSUM") as psgq,
            tc.tile_pool(name="psy2", bufs=1, space="PSUM") as psy2,
        ):
            EVA = constp.tile([128, _EVA_W], bf16)
            EVB = constp.tile([128, _EVB_W], bf16)
            EVC = constp.tile([128, _EVB_W], bf16)
            CM = constp.tile([128, _CM.shape[1]], bf16)
            CB = constp.tile([128, _CB.shape[1]], bf16)
            U = constp.tile([128, 512], bf16)
            # sync ring: critical path (EV chunks), then CB
            nc.sync.dma_start(EVA[:], eva_d[:])
            nc.sync.dma_start(EVB[:], evb_d[:])
            nc.sync.dma_start(EVC[:], evc_d[:])
            nc.sync.dma_start(CB[:], cb_d[:])
            # u alone on the gpsimd ring (needed first by the dot chain);
            # CM rides the scalar ring behind its act-table load
            nc.gpsimd.dma_start(U[:], u_d[:])
            nc.scalar.dma_start(CM[:], cm_d[:])

            # ---- PE: xps = [x; x], 16 accumulating matmuls (2 EV chunks) --
            W = _LAMB_W
            xps = psx.tile([128, 128], f32)
            for t in range(16):
                if t < 8:
                    rhs = EVA[:, W + 128 * t : W + 128 * t + 128]
                elif t < 12:
                    rhs = EVB[:, 128 * (t - 8) : 128 * (t - 8) + 128]
                else:
                    rhs = EVC[:, 128 * (t - 12) : 128 * (t - 12) + 128]
                nc.tensor.matmul(
                    xps[:],
                    EVA[:, 60 - 4 * t : 188 - 4 * t],
                    rhs,
                    start=(t == 0),
                    stop=(t == 15),
                )

            # ---- u-dots on scalar (accum-activations) + gpsimd (products),
            # keeping DVE free for the serial FFT chain ----------------------
            Uv = U[:].rearrange("p (f s) -> p s f", s=2)  # [128, 2, 256]
            R0 = sb.tile([128, 1], f32)
            Rc = sb.tile([128, 1], f32)
            R = sb.tile([128, 2], f32)
            sc0 = sb.tile([128, 256], bf16)
            codd = sb.tile([128, 256], bf16)
            nc.scalar.activation(
                sc0[:].rearrange("p (o f) -> p o f", o=1),
                Uv[:, 0:1, :],
                mybir.ActivationFunctionType.Copy,
                accum_out=R0[:],
            )
            nc.gpsimd.tensor_tensor(
                codd[:].rearrange("p (o f) -> p o f", o=1),
                Uv[:, 1:2, :],
                cm(CM, "GODD").rearrange("p (o f) -> p o f", o=1),
                Alu.mult,
            )
            nc.scalar.activation(
                sc0[:].rearrange("p (o f) -> p o f", o=1),
                codd[:].rearrange("p (o f) -> p o f", o=1),
                mybir.ActivationFunctionType.Copy,
                accum_out=R[:, 1:2],
            )
            nc.gpsimd.tensor_tensor(Rc[:], U[:, 0:1], cm(CM, "M16"), Alu.mult)
            nc.gpsimd.tensor_tensor(R[:, 0:1], R0[:], Rc[:], Alu.add)
            R2p = sb.tile([128, 8], bf16)
            nc.gpsimd.tensor_tensor(
                R2p[:].rearrange("p (j i) -> p j i", i=2),
                cm(CM, "MASK8").rearrange("p (j i) -> p j i", i=2),
                R[:].unsqueeze(1).broadcast_to((128, 4, 2)),
                Alu.mult,
            )

            # ---- x -> SBUF (bf16 lhsT for stage 1) -------------------------
            Xh = sb.tile([64, 128], bf16)
            nc.vector.tensor_copy(Xh[:], xps[0:64, :])

            # ---- FFT stage 1: YT = [Yre|Yim] = x @ [F64re|F64im] -----------
            YT = psyt.tile([128, 128], f32)
            nc.tensor.matmul(YT[:], Xh[:], cm(CM, "F64S", 64), start=True, stop=True)

            # ---- twiddle 1: P = [Yre|Yim|Yre|Yim]*WQ; Z2 = [Zim|Zre] -------
            P = sb.tile([128, 256], bf16)
            nc.vector.tensor_tensor(
                P[:].rearrange("p (r c) -> p r c", r=2),
                YT[:].unsqueeze(1).broadcast_to((128, 2, 128)),
                cb(CB, "WQ").rearrange("p (r c) -> p r c", r=2),
                Alu.mult,
            )
            Z3 = sb.tile([128, 192], bf16)
            nc.vector.tensor_tensor(Z3[:, 64:128], P[:, 0:64], P[:, 64:128], Alu.subtract)
            nc.gpsimd.tensor_tensor(Z3[:, 0:64], P[:, 128:192], P[:, 192:256], Alu.add)
            nc.vector.scalar_tensor_tensor(
                Z3[:, 128:192], P[:, 128:192], -1.0, P[:, 192:256],
                op0=Alu.mult, op1=Alu.subtract,
            )

            # ---- stage 2: XT = [Xim|Xre] (sliding windows over Z3) ---------
            XT = psxt.tile([128, 128], f32)
            nc.tensor.matmul(XT[:], cb(CB, "F128_re"), Z3[:, 0:128], start=True, stop=False)
            nc.tensor.matmul(XT[:], cb(CB, "F128_im"), Z3[:, 64:192], start=False, stop=True)

            # acr matmul fits in the PE bubble behind stage 2
            acrP = psacr.tile([128, 8], f32)
            nc.tensor.matmul(acrP[:], cm(CM, "STK"), R2p[:], start=True, stop=True)
            acrS = sb.tile([128, 8], f32)
            nc.scalar.copy(acrS[:], acrP[:])
            tmpA = sb.tile([128, 512], bf16)
            for j in range(4):
                nc.scalar.mul(
                    tmpA[:, 128 * j : 128 * j + 128], xps[:], acrS[:, 2 * j : 2 * j + 1]
                )

            # ---- symbol (doubled): Xp4 = [Xre'|Xre'|Xim'|Xim'] -------------
            Xp4 = sb.tile([128, 256], bf16)
            nc.vector.tensor_tensor(
                Xp4[:].rearrange("p (h r c) -> p h r c", h=2, r=2),
                XT[:].rearrange("p (h c) -> p h c", h=2)
                .unsqueeze(2)
                .broadcast_to((128, 2, 2, 64)),
                cb(CB, "SG4").rearrange("p (h r c) -> p h r c", h=2, r=2),
                Alu.mult,
            )

            # ---- stage 3 (c-doubled): Gq = [Gre|Gim] on 128 partitions -----
            Gq = psgq.tile([128, 256], f32)
            nc.tensor.matmul(Gq[:], Xp4[:, 0:128], cb(CB, "IA"), start=True, stop=False)
            nc.tensor.matmul(Gq[:], Xp4[:, 128:256], cb(CB, "IB"), start=False, stop=True)

            # ---- twiddle 2: Pq = [Gre|Gim|Gre|Gim]*WiQ2; Q = [Qre|Qim] -----
            Pq = sb.tile([128, 512], bf16)
            nc.vector.tensor_tensor(
                Pq[:].rearrange("p (r c) -> p r c", r=2),
                Gq[:].unsqueeze(1).broadcast_to((128, 2, 256)),
                cb(CB, "WiQ2").rearrange("p (r c) -> p r c", r=2),
                Alu.mult,
            )
            Qq = sb.tile([128, 256], bf16)
            nc.vector.tensor_tensor(Qq[:, 0:128], Pq[:, 0:128], Pq[:, 128:256], Alu.subtract)
            nc.gpsimd.tensor_tensor(Qq[:, 128:256], Pq[:, 256:384], Pq[:, 384:512], Alu.add)

            # ---- stage 4: Y2 = [w1-grid; w1-grid] --------------------------
            Y2 = psy2.tile([128, 128], f32)
            nc.tensor.matmul(
                Y2[:], cm(CM, "I64_re2", 64), Qq[0:64, 0:128], start=True, stop=False
            )
            nc.tensor.matmul(
                Y2[:], cm(CM, "I64_imN2", 64), Qq[0:64, 128:256], start=False, stop=True
            )

            # ---- final: res_j = c_j * Y2 + tmpA_j; per-pair DMA ------------
            for j in range(4):
                OUT = sb.tile([128, 128], bf16, tag=f"out{j}")
                nc.vector.scalar_tensor_tensor(
                    OUT[:],
                    Y2[:], acrS[:, 2 * j + 1 : 2 * j + 2],
                    tmpA[:, 128 * j : 128 * j + 128],
                    op0=Alu.mult, op1=Alu.add,
                )
                eng = nc.sync if j % 2 == 0 else nc.scalar
                eng.dma_start(out_d[:, 128 * j : 128 * j + 128], OUT[:])

    nc.compile()
    return nc


def _get_compiled():
    global _COMPILED
    if _COMPILED is None:
        _COMPILED = _build_nc()
    return _COMPILED


# ---------------------------------------------------------------- entry


def _make_in_maps(u, eigenvectors, eigenvalues):
    u = np.ascontiguousarray(np.asarray(u, np.float32).astype(_BF))
    # pure relayout (zero flops): EVr[32s+k, 128t+b] = ev[k, 128(4t+s)+b]
    evr = (
        np.asarray(eigenvectors, np.float32)
        .astype(_BF)
        .reshape(_K, 16, 4, 128)
        .transpose(2, 0, 1, 3)
        .reshape(128, 2048)
    )
    lamv = np.asarray(eigenvalues, np.float32).astype(_BF)
    lamb = np.zeros((128, _LAMB_W), _BF)
    for s in range(4):
        lamb[32 * s : 32 * s + 32, 60 + s] = lamv
        lamb[32 * s : 32 * s + 32, 124 + s] = lamv
    eva = np.ascontiguousarray(np.hstack([lamb, evr[:, :1024]]))
    evb = np.ascontiguousarray(evr[:, 1024:1536])
    evc = np.ascontiguousarray(evr[:, 1536:])

    in_maps = []
    for c in range(_NC):
        in_maps.append(
            {
                "u": u[c * _BS : (c + 1) * _BS].reshape(128, 512),
                "eva": eva,
                "evb": evb,
                "evc": evc,
                "cm": _CM,
                "cb": _CB,
            }
        )
    return in_maps, None


def _gather(results):
    # OUT[p, 128j + c] = out[core_batch0 + 2j + p//64, 128*(p%64) + c]
    outs = []
    for c in range(_NC):
        o = results[c]["out"].astype(np.float32).reshape(2, 64, 4, 128)
        o = o.transpose(2, 0, 1, 3).reshape(_BS, _L)  # rows 2j+h
        outs.append(o)
    return np.concatenate(outs, axis=0)


def kernel(u, eigenvectors, eigenvalues):
    from concourse.bass_utils import run_bass_kernel_spmd

    nc = _get_compiled()
    in_maps, _ = _make_in_maps(u, eigenvectors, eigenvalues)
    res = run_bass_kernel_spmd(nc, in_maps, core_ids=list(range(_NC)))
    return _gather(res.results)


# revision 19
# speedup vs baseline: 1.1334x; 1.1334x over previous
"""Trainium2 Bass kernel for nn_ConvolutionLayer (FFT conv collapse), v4.

Math: reference computes
    u_fft = rfft(u); ev_fft = rfft(ev)
    p_fft = einsum('bi,kj->bkj', u_fft, ev_fft)      # sums u_fft over i!
    conv  = irfft(p_fft); result = einsum('bkl,k->bl', conv, lam)

The einsum has no shared index, so p_fft[b,k,j] = s_b * ev_fft[k,j] with
s_b = sum_i u_fft[b,i] = u[b,:] @ g   (g = fft(indicator of first L/2+1)).
irfft is R-linear, so with s_b = a_b + i*c_b:
    result[b,:] = a_b * w0 + c_b * w1
    w0 = lam @ ev                       (since irfft(rfft(e)) = e)
    w1 = irfft(i * rfft(w0))            (by linearity over k)
w1 via a 4-step Cooley-Tukey matmul-FFT (64x128 grid), the Hermitian
symbol (+i*sgn) applied mid-chain; everything bf16 (4-5e-3 rel err vs
the 2e-2 gate).

Schedule (per core, times approximate):
  sync  ring: EVA (lam windows + ev half 1), EVB (ev half 2), CB
              (FFT constants), then out-DMAs for pairs 0/2
  gpsimd ring: CM (early constants: F64S/I64/STK/GODD/masks), U
  scalar ring: out-DMAs for pairs 1/3 (act-table load owns it early)
  PE: 16 w0 matmuls (8 per EV chunk) -> acr -> fft stages 1..4
  DVE: u-dots early; cast, twiddle1, symbol, twiddle2, 4 final STTs
  Scalar engine: acrS copy + 4 tmpA = a_j*x products off PSUM
  GpSimd engine: Zim / Qim adds (parallel with DVE's Zre / Qre)

FFT stage tricks: stage1 is 1 matmul (rhs=[F64re|F64im]); stage2 is 3
matmuls into one PSUM tile (full-width F_re pass + two half-width F_im
accumulations) so no negated operand copy is needed; symbol is one wide
multiply producing doubled windows [Xre'|Xre'|Xim'|Xim']; stage3 output
is c-doubled so twiddle2 runs on 128 partitions; stage4 rhs takes
partition slices.  Batch (64) is sharded across 8 cores, 8 rows each;
the w0/w1 pipeline is tiny and computed redundantly (no collectives).
"""

import numpy as np
import ml_dtypes

_B, _K, _L = 64, 32, 8192
_NC = 8
_BS = _B // _NC  # 8 batch rows per core

_BF = ml_dtypes.bfloat16

# ---------------------------------------------------------------- constants


def _pack(parts):
    off, offs = 0, {}
    for name, arr in parts.items():
        offs[name] = (off, arr.shape[1])
        off += arr.shape[1]
    return np.ascontiguousarray(np.concatenate(list(parts.values()), axis=1)), offs


def _build_constants():
    L, N1, N2 = _L, 64, 128
    bf = lambda x: np.ascontiguousarray(np.asarray(x, np.float32).astype(_BF))

    def pad128(a):  # place 64-row constant in rows 0:64 of a 128-row block
        out = np.zeros((128, a.shape[1]), a.dtype)
        out[:64] = a
        return out

    # g[n] = sum_{i=0}^{L/2} e^{-2pi i n i / L}; re: 1 on even n (L/2+1 at 0),
    # im: -cot(pi n / L) on odd n.  U layout: U[16b+t, f] = u[b, 512t+f].
    ind = np.zeros(L)
    ind[: L // 2 + 1] = 1.0
    g = np.fft.fft(ind)
    gim = g.imag.astype(np.float32).reshape(16, 512)  # [t, f]
    godd = np.tile(gim[:, 1::2], (8, 1))  # (128, 256)

    m16 = np.zeros((128, 1), np.float32)  # l=0 correction 4096*u[b,0]
    m16[::16, 0] = 4096.0

    mask8 = np.zeros((128, 8), np.float32)
    for p in range(128):
        j = (p // 16) // 2
        mask8[p, 2 * j : 2 * j + 2] = 1.0
    stk = np.zeros((128, 128), np.float32)
    for p in range(128):
        stk[p, 64 * ((p // 16) % 2) : 64 * ((p // 16) % 2) + 64] = 1.0

    a_i = np.arange(N1)
    b_i = np.arange(N2)
    F64 = np.exp(-2j * np.pi * np.outer(a_i, a_i) / N1)  # [a, c]
    WT = np.exp(-2j * np.pi * np.outer(b_i, a_i) / L)  # [b=n2, c=k1]
    F128 = np.exp(-2j * np.pi * np.outer(b_i, b_i) / N2)  # [b, d]
    kk = a_i[None, :] + N1 * b_i[:, None]  # spectrum index (128 d, 64 c)
    sgn = np.where(
        (kk >= 1) & (kk <= L // 2 - 1), 1.0, np.where(kk > L // 2, -1.0, 0.0)
    )
    I128 = np.exp(+2j * np.pi * np.outer(b_i, b_i) / N2)  # [d, m2]
    Wi = np.exp(+2j * np.pi * np.outer(a_i, b_i) / L)  # [c, m2]
    I64 = np.exp(+2j * np.pi * np.outer(a_i, a_i) / N1) / L  # [c, m1]

    # CM: early constants (gpsimd queue, lands first)
    pcm = {
        "I64_re2": bf(pad128(np.hstack([I64.real, I64.real]))),  # 128
        "I64_imN2": bf(pad128(np.hstack([-I64.imag, -I64.imag]))),  # 128
        "STK": bf(stk),  # 128
        "GODD": bf(godd),  # 256
        "MASK8": bf(mask8),  # 8
        "M16": bf(m16),  # 1
    }
    # CB: FFT constants (sync queue, behind the EV chunks)
    pcb = {
        "WQ": bf(np.hstack([WT.real, WT.imag, WT.imag, WT.real])),  # 256
        "F128_re": bf(F128.real),  # 128
        "F128_im": bf(F128.imag),  # 128
        "F128_imN": bf(-F128.imag),  # 128
        "SG4": bf(np.hstack([-sgn, -sgn, sgn, sgn])),  # 256
        "IA": bf(np.hstack([I128.real, I128.imag])),  # 256
        "IB": bf(np.hstack([-I128.imag, I128.real])),  # 256
        "WiQ2": bf(
            np.tile(np.hstack([Wi.real, Wi.imag, Wi.imag, Wi.real]), (2, 1))
        ),  # 512
    }
    CM, cm_off = _pack(pcm)
    CB, cb_off = _pack(pcb)
    f64s = bf(pad128(np.hstack([F64.real, F64.imag])))
    return CM, cm_off, CB, cb_off, f64s


_CM, _CM_OFF, _CB, _CB_OFF, _F64S = _build_constants()
_LAMB_W = 188
_EVA_W = _LAMB_W + 128 + 1024  # lam windows + F64S + ev cols 0:1024
_EVB_W = 512

# ---------------------------------------------------------------- bass build

_COMPILED = None


def _build_nc():
    import concourse.mybir as mybir
    import concourse.tile as tile
    from concourse import bacc

    f32 = mybir.dt.float32
    bf16 = mybir.dt.bfloat16
    Alu = mybir.AluOpType

    nc = bacc.Bacc(None)

    u_d = nc.declare_dram_parameter("u", [128, 512], bf16, isOutput=False)
    eva_d = nc.declare_dram_parameter("eva", [128, _EVA_W], bf16, isOutput=False)
    evb_d = nc.declare_dram_parameter("evb", [128, _EVB_W], bf16, isOutput=False)
    evc_d = nc.declare_dram_parameter("evc", [128, _EVB_W], bf16, isOutput=False)
    cm_d = nc.declare_dram_parameter("cm", list(_CM.shape), bf16, isOutput=False)
    cb_d = nc.declare_dram_parameter("cb", list(_CB.shape), bf16, isOutput=False)
    out_d = nc.declare_dram_parameter("out", [128, 512], bf16, isOutput=True)

    def cm(t, name, rows=None):
        off, w = _CM_OFF[name]
        return t[:, off : off + w] if rows is None else t[0:rows, off : off + w]

    def cb(t, name):
        off, w = _CB_OFF[name]
        return t[:, off : off + w]

    with tile.TileContext(nc) as tc:
        with (
            tc.tile_pool(name="const", bufs=1) as constp,
            tc.tile_pool(name="sb", bufs=1) as sb,
            tc.tile_pool(name="psx", bufs=1, space="PSUM") as psx,
            tc.tile_pool(name="psacr", bufs=1, space="PSUM") as psacr,
            tc.tile_pool(name="psyt", bufs=1, space="PSUM") as psyt,
            tc.tile_pool(name="psxt", bufs=1, space="PSUM") as psxt,
            tc.tile_pool(name="psgq", bufs=1, space="PSUM") as psgq,
            tc.tile_pool(name="psy2", bufs=1, space="PSUM") as psy2,
        ):
            EVA = constp.tile([128, _EVA_W], bf16)
            EVB = constp.tile([128, _EVB_W], bf16)
            EVC = constp.tile([128, _EVB_W], bf16)
            CM = constp.tile([128, _CM.shape[1]], bf16)
            CB = constp.tile([128, _CB.shape[1]], bf16)
            U = constp.tile([128, 512], bf16)
            # sync ring: critical path (EV chunks), then CB
            nc.sync.dma_start(EVA[:], eva_d[:])
            nc.sync.dma_start(EVB[:], evb_d[:])
            nc.sync.dma_start(EVC[:], evc_d[:])
            nc.sync.dma_start(CB[:], cb_d[:])
            # U and CM are held until EVC lands (dummy reads below) so
            # the EV chunks + CB own the DMA bandwidth early; their users
            # (u-dot chain, stage-4 tables) have ~2us of slack
            dg = sb.tile([1, 1], bf16)
            ds = sb.tile([1, 1], bf16)
            nc.gpsimd.tensor_copy(dg[:], EVC[0:1, 0:1])
            nc.gpsimd.dma_start(U[:], u_d[:])
            nc.scalar.copy(ds[:], EVC[0:1, 0:1])
            nc.scalar.dma_start(CM[:], cm_d[:])

            # ---- PE: xps = [x; x], 16 accumulating matmuls (2 EV chunks) --
            W = _LAMB_W
            xps = psx.tile([128, 128], f32)
            for t in range(16):
                if t < 8:
                    rhs = EVA[:, W + 128 + 128 * t : W + 256 + 128 * t]
                elif t < 12:
                    rhs = EVB[:, 128 * (t - 8) : 128 * (t - 8) + 128]
                else:
                    rhs = EVC[:, 128 * (t - 12) : 128 * (t - 12) + 128]
                nc.tensor.matmul(
                    xps[:],
                    EVA[:, 60 - 4 * t : 188 - 4 * t],
                    rhs,
                    start=(t == 0),
                    stop=(t == 15),
                )

            # ---- u-dots on scalar (accum-activations) + gpsimd (products),
            # keeping DVE free for the serial FFT chain ----------------------
            Uv = U[:].rearrange("p (f s) -> p s f", s=2)  # [128, 2, 256]
            R0 = sb.tile([128, 1], f32)
            Rc = sb.tile([128, 1], f32)
            R = sb.tile([128, 2], f32)
            sc0 = sb.tile([128, 256], bf16)
            codd = sb.tile([128, 256], bf16)
            nc.scalar.activation(
                sc0[:].rearrange("p (o f) -> p o f", o=1),
                Uv[:, 0:1, :],
                mybir.ActivationFunctionType.Copy,
                accum_out=R0[:],
            )
            nc.gpsimd.tensor_tensor(
                codd[:].rearrange("p (o f) -> p o f", o=1),
                Uv[:, 1:2, :],
                cm(CM, "GODD").rearrange("p (o f) -> p o f", o=1),
                Alu.mult,
            )
            nc.scalar.activation(
                sc0[:].rearrange("p (o f) -> p o f", o=1),
                codd[:].rearrange("p (o f) -> p o f", o=1),
                mybir.ActivationFunctionType.Copy,
                accum_out=R[:, 1:2],
            )
            nc.gpsimd.tensor_tensor(Rc[:], U[:, 0:1], cm(CM, "M16"), Alu.mult)
            nc.gpsimd.tensor_tensor(R[:, 0:1], R0[:], Rc[:], Alu.add)
            R2p = sb.tile([128, 8], bf16)
            nc.gpsimd.tensor_tensor(
                R2p[:].rearrange("p (j i) -> p j i", i=2),
                cm(CM, "MASK8").rearrange("p (j i) -> p j i", i=2),
                R[:].unsqueeze(1).broadcast_to((128, 4, 2)),
                Alu.mult,
            )

            # ---- x -> SBUF (bf16 lhsT for stage 1) -------------------------
            Xh = sb.tile([64, 128], bf16)
            nc.vector.tensor_copy(Xh[:], xps[0:64, :])

            # ---- FFT stage 1: YT = [Yre|Yim] = x @ [F64re|F64im] -----------
            YT = psyt.tile([128, 128], f32)
            nc.tensor.matmul(YT[:], Xh[:], EVA[0:64, W : W + 128], start=True, stop=True)

            # ---- twiddle 1: P = [Yre|Yim|Yre|Yim]*WQ; Z2 = [Zim|Zre] -------
            P = sb.tile([128, 256], bf16)
            nc.vector.tensor_tensor(
                P[:].rearrange("p (r c) -> p r c", r=2),
                YT[:].unsqueeze(1).broadcast_to((128, 2, 128)),
                cb(CB, "WQ").rearrange("p (r c) -> p r c", r=2),
                Alu.mult,
            )
            Z3 = sb.tile([128, 192], bf16)
            nc.vector.tensor_tensor(Z3[:, 64:128], P[:, 0:64], P[:, 64:128], Alu.subtract)
            nc.gpsimd.tensor_tensor(Z3[:, 0:64], P[:, 128:192], P[:, 192:256], Alu.add)
            nc.vector.scalar_tensor_tensor(
                Z3[:, 128:192], P[:, 128:192], -1.0, P[:, 192:256],
                op0=Alu.mult, op1=Alu.subtract,
            )

            # ---- stage 2: XT = [Xim|Xre] (sliding windows over Z3) ---------
            XT = psxt.tile([128, 128], f32)
            nc.tensor.matmul(XT[:], cb(CB, "F128_re"), Z3[:, 0:128], start=True, stop=False)
            nc.tensor.matmul(XT[:], cb(CB, "F128_im"), Z3[:, 64:192], start=False, stop=True)

            # acr matmul fits in the PE bubble behind stage 2
            acrP = psacr.tile([128, 8], f32)
            nc.tensor.matmul(acrP[:], cm(CM, "STK"), R2p[:], start=True, stop=True)
            acrS = sb.tile([128, 8], f32)
            nc.scalar.copy(acrS[:], acrP[:])
            tmpA = sb.tile([128, 512], bf16)
            for j in range(4):
                nc.scalar.mul(
                    tmpA[:, 128 * j : 128 * j + 128], xps[:], acrS[:, 2 * j : 2 * j + 1]
                )

            # ---- symbol (doubled): Xp4 = [Xre'|Xre'|Xim'|Xim'] -------------
            Xp4 = sb.tile([128, 256], bf16)
            nc.vector.tensor_tensor(
                Xp4[:].rearrange("p (h r c) -> p h r c", h=2, r=2),
                XT[:].rearrange("p (h c) -> p h c", h=2)
                .unsqueeze(2)
                .broadcast_to((128, 2, 2, 64)),
                cb(CB, "SG4").rearrange("p (h r c) -> p h r c", h=2, r=2),
                Alu.mult,
            )

            # ---- stage 3 (c-doubled): Gq = [Gre|Gim] on 128 partitions -----
            Gq = psgq.tile([128, 256], f32)
            nc.tensor.matmul(Gq[:], Xp4[:, 0:128], cb(CB, "IA"), start=True, stop=False)
            nc.tensor.matmul(Gq[:], Xp4[:, 128:256], cb(CB, "IB"), start=False, stop=True)

            # ---- twiddle 2: Pq = [Gre|Gim|Gre|Gim]*WiQ2; Q = [Qre|Qim] -----
            Pq = sb.tile([128, 512], bf16)
            nc.vector.tensor_tensor(
                Pq[:].rearrange("p (r c) -> p r c", r=2),
                Gq[:].unsqueeze(1).broadcast_to((128, 2, 256)),
                cb(CB, "WiQ2").rearrange("p (r c) -> p r c", r=2),
                Alu.mult,
            )
            Qq = sb.tile([128, 256], bf16)
            nc.vector.tensor_tensor(Qq[:, 0:128], Pq[:, 0:128], Pq[:, 128:256], Alu.subtract)
            nc.gpsimd.tensor_tensor(Qq[:, 128:256], Pq[:, 256:384], Pq[:, 384:512], Alu.add)

            # ---- stage 4: Y2 = [w1-grid; w1-grid] --------------------------
            Y2 = psy2.tile([128, 128], f32)
            nc.tensor.matmul(
                Y2[:], cm(CM, "I64_re2", 64), Qq[0:64, 0:128], start=True, stop=False
            )
            nc.tensor.matmul(
                Y2[:], cm(CM, "I64_imN2", 64), Qq[0:64, 128:256], start=False, stop=True
            )

            # ---- final: res_j = c_j * Y2 + tmpA_j; per-pair DMA ------------
            for j in range(4):
                OUT = sb.tile([128, 128], bf16, tag=f"out{j}")
                nc.vector.scalar_tensor_tensor(
                    OUT[:],
                    Y2[:], acrS[:, 2 * j + 1 : 2 * j + 2],
                    tmpA[:, 128 * j : 128 * j + 128],
                    op0=Alu.mult, op1=Alu.add,
                )
                eng = nc.sync if j % 2 == 0 else nc.scalar
                eng.dma_start(out_d[:, 128 * j : 128 * j + 128], OUT[:])

    nc.compile()
    return nc


def _get_compiled():
    global _COMPILED
    if _COMPILED is None:
        _COMPILED = _build_nc()
    return _COMPILED


# ---------------------------------------------------------------- entry


def _make_in_maps(u, eigenvectors, eigenvalues):
    u = np.ascontiguousarray(np.asarray(u, np.float32).astype(_BF))
    # pure relayout (zero flops): EVr[32s+k, 128t+b] = ev[k, 128(4t+s)+b]
    evr = (
        np.asarray(eigenvectors, np.float32)
        .astype(_BF)
        .reshape(_K, 16, 4, 128)
        .transpose(2, 0, 1, 3)
        .reshape(128, 2048)
    )
    lamv = np.asarray(eigenvalues, np.float32).astype(_BF)
    lamb = np.zeros((128, _LAMB_W), _BF)
    for s in range(4):
        lamb[32 * s : 32 * s + 32, 60 + s] = lamv
        lamb[32 * s : 32 * s + 32, 124 + s] = lamv
    eva = np.ascontiguousarray(np.hstack([lamb, _F64S, evr[:, :1024]]))
    evb = np.ascontiguousarray(evr[:, 1024:1536])
    evc = np.ascontiguousarray(evr[:, 1536:])

    in_maps = []
    for c in range(_NC):
        in_maps.append(
            {
                "u": u[c * _BS : (c + 1) * _BS].reshape(128, 512),
                "eva": eva,
                "evb": evb,
                "evc": evc,
                "cm": _CM,
                "cb": _CB,
            }
        )
    return in_maps, None


def _gather(results):
    # OUT[p, 128j + c] = out[core_batch0 + 2j + p//64, 128*(p%64) + c]
    outs = []
    for c in range(_NC):
        o = results[c]["out"].astype(np.float32).reshape(2, 64, 4, 128)
        o = o.transpose(2, 0, 1, 3).reshape(_BS, _L)  # rows 2j+h
        outs.append(o)
    return np.concatenate(outs, axis=0)


def kernel(u, eigenvectors, eigenvalues):
    from concourse.bass_utils import run_bass_kernel_spmd

    nc = _get_compiled()
    in_maps, _ = _make_in_maps(u, eigenvectors, eigenvalues)
    res = run_bass_kernel_spmd(nc, in_maps, core_ids=list(range(_NC)))
    return _gather(res.results)
